# revision 20
# baseline (speedup 1.0000x reference)
"""Trainium2 Bass kernel for DenseEquivariantPointAttention (IPA-style).

Sharding: queries (N=768) split across 8 cores (96 each); params and key-side
tensors replicated — each core runs full-row attention for its query shard.

Restructurings vs the reference math (all exact up to fp rounding):
  - pt_att expanded via ||qp-kp||^2 = |qp|^2 + |kp|^2 - 2<qp,kp>; the |qp|^2
    term is softmax-row-constant and dropped; -0.5*hw*|kp|^2 becomes a per-key
    column bias; hw*<qp,kp> joins q.k inside one 76-dim PE contraction.
  - softmax without max-subtraction (logits are O(10)); normalization deferred:
    unnormalized exp-weights drive all aggregations; outputs are scaled by 1/S
    afterwards (S comes from a ones-column appended to v).
  - pair_z never materialized: o_pair = ((a_un @ z[n]) @ wdz)/S + bdz.
  - is_atom blending folded into the projections by pre-scaling s columns with
    ia/(1-ia); residue+atom accumulate in PSUM. The rigid rotation commutes
    with row scaling, so it is applied to the (1-ia)-scaled residue points.
  - b = z @ wb uses the PE-transposed z tile as the stationary operand, which
    yields [key, head] orientation matching the logits layout directly.

Logits layout: [m(partitions), (h, n)] — softmax-sum and every aggregation
become PE matmuls contracting over the partition (key) axis.
"""
import sys

sys.path.insert(0, "/opt/trn_rl_repo")

import numpy as np
import ml_dtypes
from contextlib import ExitStack

import concourse.tile as tile
import concourse.mybir as mybir
from concourse import bacc
from concourse.bass_utils import run_bass_kernel_spmd
from concourse.masks import make_identity

F32 = mybir.dt.float32
BF16 = mybir.dt.bfloat16
AF = mybir.ActivationFunctionType
MUL = mybir.AluOpType.mult
ADD = mybir.AluOpType.add
SUB = mybir.AluOpType.subtract

N, CS, CZ, H, C, PQ, PV, N0, N1 = 768, 384, 128, 12, 64, 4, 8, 192, 64
CZ4 = CZ // 4
NCORES = 8
NS = N // NCORES            # 96 queries per core
QB = 24                     # query block
NBLK = NS // QB             # 4
MT = N // 128               # 6 key tiles
INF = 1e5
EPS = 1e-8
QK_SCALE = float(np.sqrt(1.0 / (3 * C)))

_CACHE = {}


def _build():
    nc = bacc.Bacc("TRN2", target_bir_lowering=False, debug=False,
                   num_devices=NCORES)

    def din(name, shape, dt=BF16):
        return nc.dram_tensor(name, shape, dt, kind="ExternalInput")

    z_d = din("z", [NS, N, CZ], F32)
    s_own_d = din("s_own", [NS, CS])
    rot_own_d = din("rot_own", [NS, 9], F32)
    trans_own_d = din("trans_own", [NS, 3], F32)
    ia_own_row_d = din("ia_own_row", [1, NS])
    ia_own_col_d = din("ia_own_col", [NS, 1], F32)
    s_d = din("s", [N, CS])
    rot_d = din("rot", [N, 9], F32)
    trans_d = din("trans", [N, 3], F32)
    maskcol_d = din("maskcol", [N], F32)
    ia_row_d = din("ia_row", [1, N])
    wq_d = din("wq", [CS, H * C])
    wkv_d = din("wkv", [CS, 2 * H * C])
    wqp_d = din("wqp", [CS, 3 * H * PQ])
    wkvp_d = din("wkvp", [CS, 3 * H * (PQ + PV)])
    wb_d = din("wb_s", [CZ, H])                 # pre-scaled sqrt(1/3)
    wdz_d = din("wdz", [CZ, CZ4])
    tfnq_ws_d = din("tfnq_ws", [N0, H * C])
    tfnq_wv_d = din("tfnq_wv", [N1, H * PQ])
    tfnkv_ws_d = din("tfnkv_ws", [N0, 2 * H * C])
    tfnkv_wv_d = din("tfnkv_wv", [N1, H * (PQ + PV)])
    tfno_ws_d = din("tfno_ws", [H * (C + PV + CZ4), N0])
    tfno_wv_d = din("tfno_wv", [H * PV, N1])
    wout_ext_d = din("wout_ext", [13 * 128, CS])    # [wout(1536); bout; 0pad]
    bq_d = din("bq", [1, H * C])
    bkv_d = din("bkv", [1, 2 * H * C])
    bqp_d = din("bqp", [1, 3 * H * PQ])
    bkvp_d = din("bkvp", [1, 3 * H * (PQ + PV)])
    bb_d = din("bb_s", [1, H])                  # pre-scaled sqrt(1/3)
    bdz_d = din("bdz", [1, CZ4])
    hw_d = din("hw_row", [1, H])
    nhw_d = din("nhw_row", [1, H])

    out_d = nc.dram_tensor("out", [NS, CS], F32, kind="ExternalOutput")

    ev_n = [0]

    def _allowed(base):
        if base % 64 == 0:
            return 128 - base
        return 32

    def evac(out_ap, in_ap):
        """psum->sbuf copy, alternating ACT/DVE to balance load"""
        ev_n[0] += 1
        if ev_n[0] % 2:
            nc.scalar.copy(out_ap, in_ap)
        else:
            nc.vector.tensor_copy(out_ap, in_ap)



    with tile.TileContext(nc) as tc, ExitStack() as ctx:
        cpool = ctx.enter_context(tc.tile_pool(name="const", bufs=1))
        spool = ctx.enter_context(tc.tile_pool(name="static", bufs=1))
        zpool = ctx.enter_context(tc.tile_pool(name="zblk", bufs=1))
        apool = ctx.enter_context(tc.tile_pool(name="ablk", bufs=1))
        work = ctx.enter_context(tc.tile_pool(name="work", bufs=1))
        ps_qk = ctx.enter_context(tc.tile_pool(name="ps_qk", bufs=1, space="PSUM"))
        ps_B = ctx.enter_context(tc.tile_pool(name="ps_B", bufs=1, space="PSUM"))
        ps_zT = ctx.enter_context(tc.tile_pool(name="ps_zT", bufs=1, space="PSUM"))
        ps_big = ctx.enter_context(tc.tile_pool(name="ps_big", bufs=1, space="PSUM"))

        def big_ps():
            return ps_big.tile([128, 2048], F32, tag="ps_big", name="bigps")

        def zt_ps(dt=BF16):
            return ps_zT.tile([128, 128], dt, tag="ps_zT", bufs=2, name="ztps")

        # ---- constants ----
        ident = cpool.tile([128, 128], BF16)
        make_identity(nc, ident[:])
        ident32 = cpool.tile([128, 128], F32)
        make_identity(nc, ident32[:])
        ones_row = cpool.tile([1, N], BF16)
        nc.vector.memset(ones_row[:], 1.0)
        eps_col = cpool.tile([128, 1], F32)
        nc.vector.memset(eps_col[:], EPS)

        def load(pool, dram, shape, dt, rearr=None, tag=""):
            t = pool.tile(shape, dt, tag=tag, name=tag or "ld")
            src = dram.ap() if rearr is None else dram.ap().rearrange(rearr, p=128)
            nc.sync.dma_start(out=t[:], in_=src)
            return t

        s_own_sb = load(spool, s_own_d, [NS, CS], BF16, tag="s_own")
        rotk = load(spool, rot_d, [128, MT, 9], F32, "(t p) c -> p t c", tag="rotk")
        transk = load(spool, trans_d, [128, MT, 3], F32, "(t p) c -> p t c", tag="transk")
        rot_own = load(spool, rot_own_d, [NS, 9], F32, tag="rot_own")
        trans_own = load(spool, trans_own_d, [NS, 3], F32, tag="trans_own")
        maskcol = load(spool, maskcol_d, [128, MT], F32, "(t p) -> p t", tag="maskcol")
        ia_own_col = load(spool, ia_own_col_d, [NS, 1], F32, tag="ia_own_col")
        ia_row = load(cpool, ia_row_d, [1, N], BF16, tag="ia_row")
        ia_own_row = load(cpool, ia_own_row_d, [1, NS], BF16, tag="ia_own_row")
        wb_sb = load(spool, wb_d, [CZ, H], BF16, tag="wb")
        wdz_sb = load(spool, wdz_d, [CZ, CZ4], BF16, tag="wdz")
        brow = {}
        for nm, d, w in [("bq", bq_d, H * C), ("bkv", bkv_d, 2 * H * C),
                         ("bqp", bqp_d, 3 * H * PQ),
                         ("bkvp", bkvp_d, 3 * H * (PQ + PV)),
                         ("bb", bb_d, H), ("bdz", bdz_d, CZ4),
                         ("hw", hw_d, H), ("nhw", nhw_d, H)]:
            brow[nm] = load(cpool, d, [1, w], BF16, tag=f"br_{nm}")
        iabar_row = cpool.tile([1, N], BF16, tag="iabar_row")
        nc.vector.tensor_scalar(iabar_row[:], ia_row[:], -1.0, 1.0, op0=MUL, op1=ADD)
        iabar_own_row = cpool.tile([1, NS], BF16, tag="iabar_own_row")
        nc.vector.tensor_scalar(iabar_own_row[:], ia_own_row[:], -1.0, 1.0,
                                op0=MUL, op1=ADD)

        # ---- z DMA: f32 dram -> bf16 sbuf (SWDGE cast), per query ----
        z_blks = []
        for b in range(NBLK):
            zt = zpool.tile([128, QB * MT * 128], BF16, tag="zblk", bufs=2)
            for q in range(QB):
                nc.gpsimd.dma_start(
                    out=zt[:, q * MT * 128:(q + 1) * MT * 128].rearrange(
                        "p (t c) -> p t c", c=128),
                    in_=z_d[b * QB + q].rearrange("(t p) c -> p t c", p=128))
            z_blks.append(zt)

        # ---- broadcast rows via PE ones outer-product ----
        def bcast_row(row_ap, ncols, rows, out_dt, tag):
            ps = big_ps()
            nc.tensor.matmul(ps[0:rows, 0:ncols], ones_row[0:1, 0:rows],
                             row_ap[0:1, 0:ncols], start=True, stop=True)
            t = cpool.tile([rows, ncols], out_dt, tag=tag, name=tag)
            evac(t[:], ps[0:rows, 0:ncols])
            return t

        ia_bc = cpool.tile([128, N], BF16, tag="ia_bc")
        ps = big_ps()
        for j in range(2):
            nc.tensor.matmul(ps[:, j * 512:j * 512 + 384], ones_row[0:1, 0:128],
                             ia_row[0:1, j * 384:(j + 1) * 384],
                             start=True, stop=True)
        for j in range(2):
            evac(ia_bc[:, j * 384:(j + 1) * 384], ps[:, j * 512:j * 512 + 384])
        iabar_bc = cpool.tile([128, N], BF16, tag="iabar_bc")
        nc.vector.tensor_scalar(iabar_bc[:], ia_bc[:], -1.0, 1.0, op0=MUL, op1=ADD)
        ia_obc = bcast_row(ia_own_row[:], NS, 128, BF16, "ia_obc")
        iabar_obc = cpool.tile([128, NS], BF16, tag="iabar_obc")
        nc.vector.tensor_scalar(iabar_obc[:], ia_obc[:], -1.0, 1.0, op0=MUL, op1=ADD)
        hw_obc = bcast_row(brow["hw"][:], H, NS, F32, "hw_obc")
        nhw_bc = bcast_row(brow["nhw"][:], H, 128, F32, "nhw_bc")
        bb_bc = bcast_row(brow["bb"][:], H, 128, F32, "bb_bc")
        bdzT = cpool.tile([CZ4, 1], F32, tag="bdzT")
        pz = zt_ps()
        nc.tensor.transpose(pz[0:CZ4, 0:1], brow["bdz"][0:1, :], ident[0:1, 0:1])
        nc.vector.tensor_copy(bdzT[:], pz[0:CZ4, 0:1])

        # ================= PHASE 0: projections =================
        khatT = spool.tile([128, H, N], BF16, tag="khatT")      # rows 0:76
        qhatT = spool.tile([128, H, NS], BF16, tag="qhatT")     # rows 0:76
        v_sb = spool.tile([128, MT, H, C + 1], BF16, tag="v_sb")
        kpts = spool.tile([128, MT, H, 3 * PQ], BF16, tag="kpts")
        vpts = spool.tile([128, MT, H, 3 * PV], BF16, tag="vpts")
        colterm = spool.tile([128, MT, H], F32, tag="colterm")
        featsT = spool.tile([128, 13, NS], BF16, tag="featsT")
        optlT = spool.tile([H * PV, 3, NS], BF16, tag="optlT")
        qpts = spool.tile([NS, H, 3 * PQ], F32, tag="qpts")
        nc.vector.memset(featsT[:, 12, :], 0.0)
        nc.vector.memset(featsT[0:1, 12, :], 1.0)

        with tc.tile_pool(name="projw", bufs=1) as wpool:
            s_sb = load(wpool, s_d, [128, MT, CS], BF16, "(t p) c -> p t c", tag="s_sb")
            wq_sb = load(wpool, wq_d, [128, 3, H * C], BF16, "(t p) c -> p t c", tag="wq")
            wkv_sb = load(wpool, wkv_d, [128, 3, 2 * H * C], BF16,
                          "(t p) c -> p t c", tag="wkv")
            wqp_sb = load(wpool, wqp_d, [128, 3, 3 * H * PQ], BF16,
                          "(t p) c -> p t c", tag="wqp")
            wkvp_sb = load(wpool, wkvp_d, [128, 3, 3 * H * (PQ + PV)], BF16,
                           "(t p) c -> p t c", tag="wkvp")
            tq_ws = wpool.tile([128, 2, H * C], BF16, tag="tq_ws")
            nc.sync.dma_start(out=tq_ws[:, 0, :], in_=tfnq_ws_d[0:128, :])
            nc.sync.dma_start(out=tq_ws[0:64, 1, :], in_=tfnq_ws_d[128:192, :])
            tkv_ws = wpool.tile([128, 2, 2 * H * C], BF16, tag="tkv_ws")
            nc.sync.dma_start(out=tkv_ws[:, 0, :], in_=tfnkv_ws_d[0:128, :])
            nc.sync.dma_start(out=tkv_ws[0:64, 1, :], in_=tfnkv_ws_d[128:192, :])
            tq_wv = load(wpool, tfnq_wv_d, [N1, H * PQ], BF16, tag="tq_wv")
            tkv_wv = load(wpool, tfnkv_wv_d, [N1, H * (PQ + PV)], BF16, tag="tkv_wv")

            # ---- transposed, ia-scaled s variants ----
            sT_r = wpool.tile([128, 3, N], BF16, tag="sT_r")
            sT_a = wpool.tile([128, 2, N], BF16, tag="sT_a")
            vecT = wpool.tile([64, 3, N], BF16, tag="vecT")
            sT_own_r = wpool.tile([128, 3, NS], BF16, tag="sT_own_r")
            sT_own_a = wpool.tile([128, 2, NS], BF16, tag="sT_own_a")
            vecT_own = wpool.tile([64, 3, NS], BF16, tag="vecT_own")
            for t in range(MT):
                sl = slice(t * 128, (t + 1) * 128)
                for kt in range(3):
                    pz = zt_ps()
                    nc.tensor.transpose(pz[:], s_sb[:, t, kt * 128:(kt + 1) * 128],
                                        ident[:])
                    nc.vector.tensor_mul(sT_r[:, kt, sl], pz[:], iabar_bc[:, sl])
                    if kt < 2:
                        rows = 128 if kt == 0 else 64
                        nc.vector.tensor_mul(sT_a[0:rows, kt, sl], pz[0:rows, :],
                                             ia_bc[0:rows, sl])
                for x in range(3):
                    pz = zt_ps()
                    nc.tensor.transpose(pz[0:64, :], s_sb[:, t, N0 + x::3], ident[:])
                    nc.vector.tensor_mul(vecT[:, x, sl], pz[0:64, :], ia_bc[0:64, sl])
            for kt in range(3):
                pz = zt_ps()
                nc.tensor.transpose(pz[0:128, 0:NS],
                                    s_own_sb[:, kt * 128:(kt + 1) * 128],
                                    ident[0:NS, 0:NS])
                nc.vector.tensor_mul(sT_own_r[:, kt, :], pz[:, 0:NS], iabar_obc[:, :])
                if kt < 2:
                    rows = 128 if kt == 0 else 64
                    nc.vector.tensor_mul(sT_own_a[0:rows, kt, :], pz[0:rows, 0:NS],
                                         ia_obc[0:rows, :])
            for x in range(3):
                pz = zt_ps()
                nc.tensor.transpose(pz[0:64, 0:NS], s_own_sb[:, N0 + x::3],
                                    ident[0:NS, 0:NS])
                nc.vector.tensor_mul(vecT_own[:, x, :], pz[0:64, 0:NS], ia_obc[0:64, :])

            # ---- kT (transposed projection) -> khatT rows 0:64 ----
            for oc in range(6):
                osl = slice(oc * 128, (oc + 1) * 128)
                ps = big_ps()
                for j in range(2):
                    psl = slice(j * 512, j * 512 + 384)
                    msl = slice(j * 384, (j + 1) * 384)
                    nc.tensor.matmul(ps[:, psl], wkv_sb[:, 0, osl], sT_r[:, 0, msl],
                                     start=True, stop=False)
                    for kt in range(1, 3):
                        nc.tensor.matmul(ps[:, psl], wkv_sb[:, kt, osl],
                                         sT_r[:, kt, msl], start=False, stop=False)
                    for kt in range(2):
                        rows = 128 if kt == 0 else 64
                        nc.tensor.matmul(ps[:, psl], tkv_ws[0:rows, kt, osl],
                                         sT_a[0:rows, kt, msl], start=False, stop=False)
                    nc.tensor.matmul(ps[:, psl], brow["bkv"][0:1, osl],
                                     iabar_row[0:1, msl], start=False, stop=True)
                for j in range(2):
                    psl = slice(j * 512, j * 512 + 384)
                    msl = slice(j * 384, (j + 1) * 384)
                    evac(khatT[0:64, 2 * oc, msl], ps[0:64, psl])
                    evac(khatT[0:64, 2 * oc + 1, msl], ps[64:128, psl])

            # ---- v (native) with appended ones column ----
            for t in range(MT):
                msl = slice(t * 128, (t + 1) * 128)
                ps = big_ps()
                for j in range(2):
                    psl = slice(j * 512, j * 512 + 384)
                    vsl = slice(H * C + j * 384, H * C + (j + 1) * 384)
                    nc.tensor.matmul(ps[:, psl], sT_r[:, 0, msl], wkv_sb[:, 0, vsl],
                                     start=True, stop=False)
                    for kt in range(1, 3):
                        nc.tensor.matmul(ps[:, psl], sT_r[:, kt, msl],
                                         wkv_sb[:, kt, vsl], start=False, stop=False)
                    for kt in range(2):
                        rows = 128 if kt == 0 else 64
                        nc.tensor.matmul(ps[:, psl], sT_a[0:rows, kt, msl],
                                         tkv_ws[0:rows, kt, vsl], start=False, stop=False)
                    nc.tensor.matmul(ps[:, psl], iabar_row[0:1, msl],
                                     brow["bkv"][0:1, vsl], start=False, stop=True)
                for j in range(2):
                    evac(v_sb[:, t, 6 * j:6 * j + 6, 0:C],
                         ps[:, j * 512:j * 512 + 384].rearrange(
                             "p (h c) -> p h c", c=C))
                nc.vector.memset(v_sb[:, t, :, C], 1.0)

            # ---- kv_pts: residue (to be rotated) + atom, separate regions ----
            for t in range(MT):
                msl = slice(t * 128, (t + 1) * 128)
                ps = big_ps()
                # residue pre-rotation part at cols 0:432
                nc.tensor.matmul(ps[:, 0:432], sT_r[:, 0, msl], wkvp_sb[:, 0, :],
                                 start=True, stop=False)
                for kt in range(1, 3):
                    nc.tensor.matmul(ps[:, 0:432], sT_r[:, kt, msl],
                                     wkvp_sb[:, kt, :], start=False, stop=False)
                nc.tensor.matmul(ps[:, 0:432], iabar_row[0:1, msl],
                                 brow["bkvp"][0:1, :], start=False, stop=True)
                # atom part at cols 512:944
                for x in range(3):
                    nc.tensor.matmul(ps[:, 512 + x * 144:512 + (x + 1) * 144],
                                     vecT[:, x, msl], tkv_wv[:, :],
                                     start=(x == 0), stop=(x == 2))
                # rotate residue, add atom + trans, cast
                acc = work.tile([128, H * (PQ + PV)], F32, tag="rigacc", bufs=2)
                tmp = work.tile([128, H * (PQ + PV)], F32, tag="rigtmp", bufs=2)
                for x in range(3):
                    nc.vector.tensor_scalar(acc[:], ps[:, 0:144],
                                            rotk[:, t, 3 * x + 0:3 * x + 1], None, op0=MUL)
                    nc.vector.scalar_tensor_tensor(acc[:], ps[:, 144:288],
                                                   rotk[:, t, 3 * x + 1:3 * x + 2], acc[:],
                                                   op0=MUL, op1=ADD)
                    nc.vector.scalar_tensor_tensor(acc[:], ps[:, 288:432],
                                                   rotk[:, t, 3 * x + 2:3 * x + 3], acc[:],
                                                   op0=MUL, op1=ADD)
                    nc.scalar.copy(tmp[:], ps[:, 512 + x * 144:512 + (x + 1) * 144])
                    nc.vector.tensor_add(acc[:], acc[:], tmp[:])
                    accv = acc[:].rearrange("p (h q) -> p h q", q=PQ + PV)
                    nc.vector.tensor_scalar(kpts[:, t, :, x * PQ:(x + 1) * PQ],
                                            accv[:, :, 0:PQ],
                                            transk[:, t, x:x + 1], None, op0=ADD)
                    nc.vector.tensor_scalar(vpts[:, t, :, x * PV:(x + 1) * PV],
                                            accv[:, :, PQ:PQ + PV],
                                            transk[:, t, x:x + 1], None, op0=ADD)

            # ---- k_pts transposes -> khatT rows 64:76 ----
            for t in range(MT):
                for h in range(H):
                    pz = zt_ps()
                    nc.tensor.transpose(pz[0:12, :], kpts[:, t, h, :], ident[:])
                    evac(khatT[64:76, h, t * 128:(t + 1) * 128], pz[0:12, :])

            # ---- Ksq -> colterm ----
            sq = work.tile([128, H, 3 * PQ], F32, tag="sq", bufs=2)
            ksq = work.tile([128, H], F32, tag="ksq", bufs=2)
            for t in range(MT):
                kp = kpts[:, t, :, :]
                nc.vector.tensor_mul(sq[:], kp, kp)
                nc.vector.tensor_reduce(ksq[:], sq[:], mybir.AxisListType.X, ADD)
                nc.vector.tensor_mul(colterm[:, t, :], ksq[:], nhw_bc[:, :])
                nc.vector.tensor_add(colterm[:, t, :], colterm[:, t, :], bb_bc[:, :])
                nc.vector.tensor_scalar(colterm[:, t, :], colterm[:, t, :],
                                        maskcol[:, t:t + 1], None, op0=ADD)

            # ---- qhatT rows 0:64 (transposed projection, scaled) ----
            for oc in range(6):
                osl = slice(oc * 128, (oc + 1) * 128)
                ps = big_ps()
                nc.tensor.matmul(ps[:, 0:NS], wq_sb[:, 0, osl], sT_own_r[:, 0, :],
                                 start=True, stop=False)
                for kt in range(1, 3):
                    nc.tensor.matmul(ps[:, 0:NS], wq_sb[:, kt, osl],
                                     sT_own_r[:, kt, :], start=False, stop=False)
                for kt in range(2):
                    rows = 128 if kt == 0 else 64
                    nc.tensor.matmul(ps[:, 0:NS], tq_ws[0:rows, kt, osl],
                                     sT_own_a[0:rows, kt, :], start=False, stop=False)
                nc.tensor.matmul(ps[:, 0:NS], brow["bq"][0:1, osl],
                                 iabar_own_row[0:1, :], start=False, stop=True)
                nc.scalar.mul(qhatT[0:64, 2 * oc, :], ps[0:64, 0:NS], QK_SCALE)
                nc.scalar.mul(qhatT[0:64, 2 * oc + 1, :], ps[64:128, 0:NS], QK_SCALE)

            # ---- q_pts native + rigid + hw scale -> qhatT rows 64:76 ----
            ps = big_ps()
            nc.tensor.matmul(ps[0:NS, 0:144], sT_own_r[:, 0, :], wqp_sb[:, 0, :],
                             start=True, stop=False)
            for kt in range(1, 3):
                nc.tensor.matmul(ps[0:NS, 0:144], sT_own_r[:, kt, :],
                                 wqp_sb[:, kt, :], start=False, stop=False)
            nc.tensor.matmul(ps[0:NS, 0:144], iabar_own_row[0:1, :],
                             brow["bqp"][0:1, :], start=False, stop=True)
            nc.tensor.matmul(ps[0:NS, 512:560], vecT_own[:, 0, :], tq_wv[:, :],
                             start=True, stop=False)
            for x in range(1, 3):
                nc.tensor.matmul(ps[0:NS, 512 + x * 48:512 + (x + 1) * 48],
                                 vecT_own[:, x, :], tq_wv[:, :],
                                 start=False, stop=(x == 2))
            for x in range(3):
                acc = work.tile([NS, H * PQ], F32, tag="qrigacc", bufs=2)
                tmp = work.tile([NS, H * PQ], F32, tag="qrigtmp", bufs=2)
                nc.vector.tensor_scalar(acc[:], ps[0:NS, 0:48],
                                        rot_own[:, 3 * x + 0:3 * x + 1], None, op0=MUL)
                nc.vector.scalar_tensor_tensor(acc[:], ps[0:NS, 48:96],
                                               rot_own[:, 3 * x + 1:3 * x + 2], acc[:],
                                               op0=MUL, op1=ADD)
                nc.vector.scalar_tensor_tensor(acc[:], ps[0:NS, 96:144],
                                               rot_own[:, 3 * x + 2:3 * x + 3], acc[:],
                                               op0=MUL, op1=ADD)
                nc.scalar.copy(tmp[:], ps[0:NS, 512 + x * 48:512 + (x + 1) * 48])
                nc.vector.tensor_add(acc[:], acc[:], tmp[:])
                nc.vector.tensor_scalar(qpts[:, :, x * PQ:(x + 1) * PQ],
                                        acc[:].rearrange("p (h q) -> p h q", q=PQ),
                                        trans_own[:, x:x + 1], None, op0=ADD)
            for h in range(H):
                nc.vector.tensor_scalar(qpts[:, h, :], qpts[:, h, :],
                                        hw_obc[:, h:h + 1], None, op0=MUL)
            for h in range(H):
                pz = zt_ps(F32)
                nc.tensor.transpose(pz[0:12, 0:NS], qpts[:, h, :],
                                    ident32[0:NS, 0:NS])
                evac(qhatT[64:76, h, :], pz[0:12, 0:NS])

        # ================= per-block L / A / P phases =================
        for b in range(NBLK):
            bsl = slice(b * QB, (b + 1) * QB)
            a_blk = apool.tile([128, MT, H, QB], BF16, tag="ablk", bufs=2)
            rot_blk = work.tile([QB, 9], F32, tag="rot_blk", bufs=2)
            nc.sync.dma_start(out=rot_blk[:], in_=rot_own_d[b * QB:(b + 1) * QB, :])
            trans_blk = work.tile([QB, 3], F32, tag="trans_blk", bufs=2)
            nc.sync.dma_start(out=trans_blk[:], in_=trans_own_d[b * QB:(b + 1) * QB, :])
            invS = work.tile([QB, H], F32, tag="invS", bufs=2)
            invS_T = work.tile([H, QB], F32, tag="invS_T", bufs=2)
            PT_sb = work.tile([128, QB * H], BF16, tag="PTsb", bufs=2)

            # ---- L: logits + exp, per key tile ----
            for mt in range(MT):
                msl = slice(mt * 128, (mt + 1) * 128)
                B_ps = ps_B.tile([128, QB * 16], F32, tag="ps_B")
                for q in range(QB):
                    pz = zt_ps()
                    nc.tensor.transpose(
                        pz[:], z_blks[b][:, (q * MT + mt) * 128:(q * MT + mt + 1) * 128],
                        ident[:])
                    zT_sb = work.tile([128, 128], BF16, tag="zT", bufs=4)
                    evac(zT_sb[:], pz[:])
                    nc.tensor.matmul(B_ps[:, q * 16:q * 16 + H], zT_sb[:], wb_sb[:],
                                     start=(q == 0), stop=(q == QB - 1))
                qk_ps = ps_qk.tile([128, H * 32], F32, tag="ps_qk")
                for h in range(H):
                    nc.tensor.matmul(qk_ps[:, h * 32:h * 32 + QB],
                                     khatT[0:76, h, msl], qhatT[0:76, h, bsl],
                                     start=(h == 0), stop=(h == H - 1))
                B_sb = work.tile([128, QB * 16], BF16, tag="Bsb", bufs=2)
                nc.scalar.copy(B_sb[:], B_ps[:])
                Lsum = work.tile([128, H, QB], F32, tag="Lsum", bufs=2)
                nc.vector.tensor_add(
                    Lsum[:],
                    qk_ps[:].rearrange("p (h n) -> p h n", n=32)[:, :, 0:QB],
                    B_sb[:].rearrange("p (n h) -> p h n", h=16)[:, 0:H, :])
                nc.vector.tensor_add(
                    Lsum[:], Lsum[:],
                    colterm[:, mt, :].broadcast_to((128, H, QB)))
                nc.scalar.activation(a_blk[:, mt, :, :], Lsum[:], AF.Exp)

            # ---- A: aggregate o / o_pt / S ----
            agg = big_ps()
            for mt in range(MT):
                st, sp = mt == 0, mt == MT - 1
                for h in range(H):
                    nc.tensor.matmul(agg[0:QB, h * 128:h * 128 + C + 1],
                                     a_blk[:, mt, h, :], v_sb[:, mt, h, :],
                                     start=(st and h % 4 == 0), stop=(sp and h % 4 == 3))
                for h in range(H):
                    nc.tensor.matmul(
                        agg[0:QB, 1536 + h * 32:1536 + h * 32 + 3 * PV],
                        a_blk[:, mt, h, :], vpts[:, mt, h, :],
                        start=(st and h == 0), stop=(sp and h == H - 1))
            nc.vector.tensor_copy(invS[:], agg[0:QB, C::128][:, 0:H])
            nc.vector.reciprocal(invS[:], invS[:])
            pz = zt_ps(F32)
            nc.tensor.transpose(pz[0:H, 0:QB], invS[:], ident32[0:QB, 0:QB])
            nc.vector.tensor_copy(invS_T[:], pz[0:H, 0:QB])

            # o -> featsT rows 0:768 (transpose per head)
            for h in range(H):
                o_tmp = work.tile([QB, C], BF16, tag="o_tmp", bufs=3)
                nc.vector.tensor_scalar(o_tmp[:], agg[0:QB, h * 128:h * 128 + C],
                                        invS[:, h:h + 1], None, op0=MUL)
                pz = zt_ps()
                nc.tensor.transpose(pz[0:C, 0:QB], o_tmp[:], ident[0:QB, 0:QB])
                evac(featsT[(h % 2) * C:(h % 2 + 1) * C, h // 2, bsl], pz[0:C, 0:QB])

            # o_pt: normalize, inverse rigid, norm; -> featsT rows 768:1152
            opt_un = work.tile([QB, 3, H * PV], F32, tag="opt_un", bufs=2)
            for h in range(H):
                nc.vector.tensor_scalar(
                    opt_un[:, :, h * PV:(h + 1) * PV],
                    agg[0:QB, 1536 + h * 32:1536 + h * 32 + 3 * PV].rearrange(
                        "p (x v) -> p x v", x=3),
                    invS[:, h:h + 1], None, op0=MUL)
            for j in range(3):
                nc.vector.tensor_scalar(opt_un[:, j, :], opt_un[:, j, :],
                                        trans_blk[:, j:j + 1], None, op0=SUB)
            optl = work.tile([QB, 3, H * PV], F32, tag="optl", bufs=2)
            for i in range(3):
                nc.vector.tensor_scalar(optl[:, i, :], opt_un[:, 0, :],
                                        rot_blk[:, 0 + i:1 + i], None, op0=MUL)
                nc.vector.scalar_tensor_tensor(optl[:, i, :], opt_un[:, 1, :],
                                               rot_blk[:, 3 + i:4 + i], optl[:, i, :],
                                               op0=MUL, op1=ADD)
                nc.vector.scalar_tensor_tensor(optl[:, i, :], opt_un[:, 2, :],
                                               rot_blk[:, 6 + i:7 + i], optl[:, i, :],
                                               op0=MUL, op1=ADD)
            nrm = work.tile([QB, H * PV], F32, tag="nrm", bufs=2)
            t0 = work.tile([QB, H * PV], F32, tag="nrm_t0", bufs=2)
            nc.vector.tensor_mul(nrm[:], optl[:, 0, :], optl[:, 0, :])
            nc.vector.tensor_mul(t0[:], optl[:, 1, :], optl[:, 1, :])
            nc.vector.tensor_add(nrm[:], nrm[:], t0[:])
            nc.vector.tensor_mul(t0[:], optl[:, 2, :], optl[:, 2, :])
            nc.vector.tensor_add(nrm[:], nrm[:], t0[:])
            nc.scalar.activation(nrm[:], nrm[:], AF.Sqrt, bias=eps_col[0:QB, 0:1])
            # transposes into featsT (rows 768..1152 span tiles 6,7,8)
            for xi in range(4):
                srcap = optl[:, xi, :] if xi < 3 else nrm[:]
                pz = zt_ps(F32)
                nc.tensor.transpose(pz[0:H * PV, 0:QB], srcap, ident32[0:QB, 0:QB])
                gr0 = 768 + xi * 96         # global feats row of psum row 0
                r = 0
                while r < 96:
                    gr = gr0 + r
                    tno, ro = gr // 128, gr % 128
                    step = min(96 - r, _allowed(ro), _allowed(r), 128 - ro)
                    evac(featsT[ro:ro + step, tno, bsl], pz[r:r + step, 0:QB])
                    r += step
                if xi < 3:
                    evac(optlT[:, xi, bsl], pz[0:H * PV, 0:QB])

            # ---- P: o_pair path ----
            for sblk in range(QB // 12):
                P_ps = big_ps()
                for ql in range(12):
                    q = sblk * 12 + ql
                    for mt in range(MT):
                        nc.tensor.matmul(
                            P_ps[0:H, ql * 128:(ql + 1) * 128],
                            a_blk[:, mt, :, q], z_blks[b][:, (q * MT + mt) * 128:
                                                          (q * MT + mt + 1) * 128],
                            start=(mt == 0 and ql % 4 == 0),
                            stop=(mt == MT - 1 and ql % 4 == 3))
                for ql in range(12):
                    q = sblk * 12 + ql
                    P_sb = work.tile([H, 128], BF16, tag="P_sb", bufs=3)
                    nc.vector.tensor_scalar(P_sb[:], P_ps[0:H, ql * 128:(ql + 1) * 128],
                                            invS_T[:, q:q + 1], None, op0=MUL)
                    pz = zt_ps()
                    nc.tensor.transpose(pz[:, 0:H], P_sb[:], ident[0:H, 0:H])
                    evac(PT_sb[:, q * H:(q + 1) * H], pz[:, 0:H])
            op_ps = ps_B.tile([128, QB * 16], F32, tag="ps_B")
            for h in range(H):
                nc.tensor.matmul(op_ps[0:CZ4, h * QB:(h + 1) * QB], wdz_sb[:, :],
                                 PT_sb[:, h::H], start=(h == 0), stop=(h == H - 1))
            for h in range(H):
                nc.vector.tensor_scalar(
                    featsT[(h % 4) * CZ4:(h % 4 + 1) * CZ4, 9 + h // 4, bsl],
                    op_ps[0:CZ4, h * QB:(h + 1) * QB], bdzT[:, 0:1], None, op0=ADD)

        # ================= output heads =================
        with tc.tile_pool(name="outw", bufs=1) as opool:
            wout_sb = load(opool, wout_ext_d, [128, 13, CS], BF16,
                           "(t p) c -> p t c", tag="wout")
            tfno_sb = opool.tile([128, 10, N0], BF16, tag="tfno")
            for t in range(6):
                nc.sync.dma_start(out=tfno_sb[:, t, :],
                                  in_=tfno_ws_d[t * 128:(t + 1) * 128, :])
            nc.sync.dma_start(out=tfno_sb[32:128, 6, :], in_=tfno_ws_d[768:864, :])
            for jj in range(3):
                nc.sync.dma_start(out=tfno_sb[:, 7 + jj, :],
                                  in_=tfno_ws_d[864 + jj * 128:864 + (jj + 1) * 128, :])
            tfnov_sb = load(opool, tfno_wv_d, [H * PV, N1], BF16, tag="tfnov")

            res_ps = big_ps()
            for t in range(13):
                nc.tensor.matmul(res_ps[0:NS, 0:CS], featsT[:, t, :], wout_sb[:, t, :],
                                 start=(t == 0), stop=(t == 12))
            res_sb = opool.tile([NS, CS], F32, tag="res_sb")
            nc.scalar.copy(res_sb[:], res_ps[0:NS, 0:CS])

            asc_ps = big_ps()
            for t in range(6):
                nc.tensor.matmul(asc_ps[0:NS, 0:N0], featsT[:, t, :], tfno_sb[:, t, :],
                                 start=(t == 0), stop=False)
            nc.tensor.matmul(asc_ps[0:NS, 0:N0], featsT[32:64, 8, :],
                             tfno_sb[32:64, 6, :], start=False, stop=False)
            nc.tensor.matmul(asc_ps[0:NS, 0:N0], featsT[64:128, 8, :],
                             tfno_sb[64:128, 6, :], start=False, stop=False)
            for jj in range(3):
                nc.tensor.matmul(asc_ps[0:NS, 0:N0], featsT[:, 9 + jj, :],
                                 tfno_sb[:, 7 + jj, :], start=False, stop=(jj == 2))
            # atom_vec at cols 512:704 (separate psum region)
            for x in range(3):
                nc.tensor.matmul(asc_ps[0:NS, 512 + x * N1:512 + (x + 1) * N1],
                                 optlT[:, x, :], tfnov_sb[:, :],
                                 start=(x == 0), stop=(x == 2))

            atom_sb = opool.tile([NS, CS], F32, tag="atom_sb")
            nc.vector.tensor_copy(atom_sb[:, 0:N0], asc_ps[0:NS, 0:N0])
            for x in range(3):
                nc.vector.tensor_copy(
                    atom_sb[:, N0 + x::3],
                    asc_ps[0:NS, 512 + x * N1:512 + (x + 1) * N1])

            d = opool.tile([NS, CS], F32, tag="d_sb")
            nc.vector.tensor_sub(d[:], atom_sb[:], res_sb[:])
            nc.vector.tensor_scalar(d[:], d[:], ia_own_col[:, 0:1], None, op0=MUL)
            out_sb = opool.tile([NS, CS], F32, tag="out_sb")
            nc.vector.tensor_add(out_sb[:], d[:], res_sb[:])
            nc.sync.dma_start(out=out_d[:, :], in_=out_sb[:])

    nc.compile()
    return nc


def _prep_inputs(inputs):
    bf = ml_dtypes.bfloat16
    f32 = np.float32

    def b16(a):
        return np.ascontiguousarray(np.asarray(a, f32).astype(bf))

    s = np.asarray(inputs["s"], f32)
    z = np.ascontiguousarray(np.asarray(inputs["z"], f32))
    rot = np.asarray(inputs["rot"], f32).reshape(N, 9)
    trans = np.asarray(inputs["trans"], f32)
    mask = np.asarray(inputs["mask"], f32)
    ia = np.asarray(inputs["is_atom"]).astype(f32)

    sq3 = f32(np.sqrt(1.0 / 3.0))
    hw = (np.logaddexp(0.0, np.asarray(inputs["head_weights"], np.float64))
          * np.sqrt(1.0 / (3 * (PQ * 9.0 / 2)))).astype(f32)

    wout_ext = np.zeros((13 * 128, CS), f32)
    wout_ext[:1536] = np.asarray(inputs["wout"], f32)
    wout_ext[1536] = np.asarray(inputs["bout"], f32)

    def kv_perm(w):
        # reference packs kv per head [H, 2, C]; kernel wants [k-block | v-block]
        w = np.asarray(w, f32).reshape(-1, H, 2, C)
        return np.concatenate([w[:, :, 0].reshape(w.shape[0], H * C),
                               w[:, :, 1].reshape(w.shape[0], H * C)], axis=1)

    rep = dict(
        s=b16(s), rot=rot, trans=trans,
        maskcol=(INF * (mask - 1.0)).astype(f32),
        ia_row=b16(ia[None, :]),
        wq=b16(inputs["wq"]), wkv=b16(kv_perm(inputs["wkv"])), wqp=b16(inputs["wqp"]),
        wkvp=b16(inputs["wkvp"]),
        wb_s=b16(np.asarray(inputs["wb"], f32) * sq3),
        wdz=b16(inputs["wdz"]),
        tfnq_ws=b16(inputs["tfnq_ws"]), tfnq_wv=b16(inputs["tfnq_wv"]),
        tfnkv_ws=b16(kv_perm(inputs["tfnkv_ws"])), tfnkv_wv=b16(inputs["tfnkv_wv"]),
        tfno_ws=b16(inputs["tfno_ws"]), tfno_wv=b16(inputs["tfno_wv"]),
        wout_ext=b16(wout_ext),
        bq=b16(np.asarray(inputs["bq"], f32)[None, :]),
        bkv=b16(kv_perm(np.asarray(inputs["bkv"], f32)[None, :])),
        bqp=b16(np.asarray(inputs["bqp"], f32)[None, :]),
        bkvp=b16(np.asarray(inputs["bkvp"], f32)[None, :]),
        bb_s=b16(np.asarray(inputs["bb"], f32)[None, :] * sq3),
        bdz=b16(np.asarray(inputs["bdz"], f32)[None, :]),
        hw_row=b16(hw[None, :]),
        nhw_row=b16((-0.5 * hw)[None, :]),
    )
    in_maps = []
    for i in range(NCORES):
        sl = slice(i * NS, (i + 1) * NS)
        m = dict(rep)
        m["z"] = z[sl]
        m["s_own"] = b16(s[sl])
        m["rot_own"] = np.ascontiguousarray(rot[sl])
        m["trans_own"] = np.ascontiguousarray(trans[sl])
        m["ia_own_row"] = b16(ia[None, sl])
        m["ia_own_col"] = np.ascontiguousarray(ia[sl, None])
        in_maps.append(m)
    return in_maps


def kernel(**inputs):
    if "nc" not in _CACHE:
        _CACHE["nc"] = _build()
    nc = _CACHE["nc"]
    in_maps = _prep_inputs(inputs)
    res = run_bass_kernel_spmd(nc, in_maps, core_ids=list(range(NCORES)))
    return np.concatenate([res.results[i]["out"] for i in range(NCORES)], axis=0)
